# revision 17
# baseline (speedup 1.0000x reference)
"""Multi-head self-attention (B=8, T=1024, D=768, H=12) on 8 Trainium2 NeuronCores.

Strategy: data-parallel over batch (1 batch element per core). Per core, all
tensors are kept in "feature-major" (transposed) layout so no on-chip
transposes are ever needed:

  phase 1:  Q^T = W_q^T @ x^T   [768,1024]   (Q pre-scaled by 1/sqrt(hd) on host)
            K^T = W_k^T @ x^T   [768,1024]
            V   = x @ W_v_aug   [1024, 12*65] natural layout, augmented with a
                  ones column per head (gives softmax denominator for free)
  phase 2:  per head pair (row-tiled on the PE array, 2 heads concurrently):
            S^T[k,q] = K^T_h.T(slice) @ Q^T_h   -> exp on ACT -> P^T
            y_aug^T = V_aug^T @ P^T  (row 64 = colsum of exp = softmax denom)
            y^T = y_unnorm^T * (1/colsum) broadcast via K=1 PE matmul
  phase 3:  out = y^T.T @ W_proj + b_proj  (bias seeded into PSUM via K=1
            ones-matmul), DMA PSUM -> DRAM.

All matmuls use the float32r dtype (full fp32 storage; PE processes it at
1 cycle/row when the moving free dim is >= 256, i.e. 4x faster than plain
fp32 matmul).
"""

import numpy as np

import concourse.bass as bass
import concourse.tile as tile
from concourse import bacc
from concourse import mybir
from concourse.bass_utils import run_bass_kernel_spmd

D = 768          # model dim
T = 1024         # sequence length
H = 12           # heads
HD = 64          # head dim
B = 8            # batch (== number of cores)
KC = D // 128    # 6 contraction chunks of 128
TC = T // 128    # 8 sequence chunks of 128
HP = H // 2      # 6 head pairs
VW = H * (HD + 1)  # 780: V augmented with a ones column per head

F32 = mybir.dt.float32
F32R = mybir.dt.float32r
EXP = mybir.ActivationFunctionType.Exp
ADD = mybir.AluOpType.add
MULT = mybir.AluOpType.mult

TRACE = False          # set True (from test harness) to collect an NTFF profile
LAST_RESULTS = None    # BassKernelResults of the most recent run (for test harness)


def _r(ap):
    return ap


def _build():
    nc = bacc.Bacc("TRN2", target_bir_lowering=False, debug=False, num_devices=B)

    xT = nc.dram_tensor("xT", [D, T], F32R, kind="ExternalInput")
    wqk = nc.dram_tensor("wqk", [D, 2 * D], F32R, kind="ExternalInput")
    bqk = nc.dram_tensor("bqk", [2 * D, 1], F32, kind="ExternalInput")
    wv = nc.dram_tensor("wv", [D, VW], F32R, kind="ExternalInput")
    bv = nc.dram_tensor("bv", [1, VW + 128], F32R, kind="ExternalInput")
    wp = nc.dram_tensor("wp", [D, D], F32R, kind="ExternalInput")
    bp = nc.dram_tensor("bp", [1, D], F32R, kind="ExternalInput")
    out = nc.dram_tensor("out", [T, D], F32, kind="ExternalOutput")

    with tile.TileContext(nc) as tc:
        with (
            tc.tile_pool(name="persist", bufs=1) as persist,
            tc.tile_pool(name="work", bufs=1) as work,
            tc.tile_pool(name="ps", bufs=1, space="PSUM") as ps,
        ):
            # ---- persistent SBUF tensors (live through phases 1-2) ----
            qk_sb = [
                persist.tile([128, T], F32R, tag=f"qk{m}", name=f"qk{m}")
                for m in range(2 * KC)
            ]  # 0..5 = Q^T tiles, 6..11 = K^T tiles
            v_sb = [
                persist.tile([128, VW], F32R, tag=f"v{t}", name=f"v{t}")
                for t in range(TC)
            ]
            bqk_sb = persist.tile([128, 2 * KC], F32, tag="bqk", name="bqk_sb")
            bv_sb = persist.tile([1, VW + 128], F32R, tag="bv", name="bv_sb")
            ones_sb = bv_sb[:, VW : VW + 128]

            nc.sync.dma_start(bv_sb[:], bv[:, :])
            # all 12 bias columns in one strided DMA: bqk[m*128+p] -> [p, m]
            nc.sync.dma_start(
                bqk_sb[:, :],
                bqk.rearrange("(m p) 1 -> p m", p=128),
            )

            def emit_v_chunk(t, xt, wv_t):
                sv = ps.tile([128, VW], F32, tag="sp", bufs=2, name=f"sp_v{t}")
                for half, (n0, n1) in enumerate(((0, 512), (512, VW))):
                    nc.tensor.matmul(
                        sv[:, n0:n1],
                        _r(ones_sb),
                        _r(bv_sb[:, n0:n1]),
                        start=True,
                        stop=False,
                    )
                    for k in range(KC):
                        nc.tensor.matmul(
                            sv[:, n0:n1],
                            _r(xt[k][:, t * 128 : (t + 1) * 128]),
                            _r(wv_t[k][:, n0:n1]),
                            start=False,
                            stop=(k == KC - 1),
                        )
                nc.vector.tensor_copy(v_sb[t][:], sv[:])

            def emit_qk_tile(m, xt, w_t, col, bcol):
                # two 1-bank PSUM halves on the "pv" tag, which is free
                # between attention pairs -- keeps the "sp" ring a pure
                # PE<->ACT pipeline during the pair chunk loops
                for nh in range(2):
                    sp = ps.tile([128, 512], F32, tag="pv", bufs=4,
                                 name=f"sp_qk{m}_{nh}")
                    for k in range(KC):
                        nc.tensor.matmul(
                            sp[:, :],
                            _r(w_t[k][:, col : col + 128]),
                            _r(xt[k][:, nh * 512 : (nh + 1) * 512]),
                            start=(k == 0),
                            stop=(k == KC - 1),
                        )
                    nc.vector.tensor_scalar_add(
                        qk_sb[m][:, nh * 512 : (nh + 1) * 512],
                        sp[:],
                        bqk_sb[:, bcol : bcol + 1],
                    )

            def emit_pair(hp_i, yt_sb):
                qt = qk_sb[hp_i]
                kt = qk_sb[KC + hp_i]
                pv = {}
                for hi in range(2):
                    for half in range(2):
                        pv[hi, half] = ps.tile(
                            [65, 512], F32, tag="pv", bufs=4,
                            name=f"pv{hp_i}_{hi}_{half}",
                        )
                for kb in range(TC):
                    sps = [
                        ps.tile([128, T], F32, tag="sp", bufs=2,
                                name=f"sp_s{hp_i}_{kb}_{hi}")
                        for hi in range(2)
                    ]
                    # QK^T: two heads concurrently in disjoint 64-row strips
                    for half in range(2):
                        for hi in range(2):
                            r0, r1 = hi * 64, (hi + 1) * 64
                            nc.tensor.matmul(
                                sps[hi][:, half * 512 : (half + 1) * 512],
                                _r(kt[r0:r1, kb * 128 : (kb + 1) * 128]),
                                _r(qt[r0:r1, half * 512 : (half + 1) * 512]),
                                start=True,
                                stop=True,
                                tile_position=(hi * 64, 0),
                            )
                    for hi in range(2):
                        h = 2 * hp_i + hi
                        p_t = work.tile([128, T], F32R, tag="p", bufs=3,
                                        name=f"p{hp_i}_{kb}_{hi}")
                        nc.scalar.activation(p_t[:], sps[hi][:], EXP)
                        for half in range(2):
                            nc.tensor.matmul(
                                pv[hi, half][:, :],
                                _r(v_sb[kb][:, h * 65 : (h + 1) * 65]),
                                _r(p_t[:, half * 512 : (half + 1) * 512]),
                                start=(kb == 0),
                                stop=(kb == TC - 1),
                            )
                # normalize: y^T[f, q] * (1/colsum[q]) and write into y^T sbuf.
                # Single copy frees the pv PSUM slot immediately; the rest of
                # the chain runs off SBUF and overlaps the next pair.
                for hi in range(2):
                    for half in range(2):
                        yc = work.tile([65, 512], F32, tag="yc", bufs=4,
                                       name=f"yc{hp_i}_{hi}_{half}")
                        nc.vector.tensor_copy(yc[:], pv[hi, half][:, :])
                        rc = work.tile([1, 512], F32R, tag="rc", bufs=2,
                                       name=f"rc{hp_i}_{hi}_{half}")
                        with nc.allow_low_precision(reason="f32r rounding of softmax denom"):
                            nc.vector.reciprocal(rc[:], yc[64:65, :])
                        rb = ps.tile([64, 512], F32, tag="pv", bufs=4,
                                     name=f"rb{hp_i}_{hi}_{half}")
                        nc.tensor.matmul(
                            rb[:, :],
                            _r(ones_sb[:, 0:64]),
                            _r(rc[:, :]),
                            start=True,
                            stop=True,
                        )
                        nc.vector.tensor_tensor(
                            yt_sb[hp_i][hi * 64 : (hi + 1) * 64,
                                        half * 512 : (half + 1) * 512],
                            yc[0:64, :],
                            rb[:, :],
                            op=MULT,
                        )

            # ---- phases 2+3 pool opened before ph1 so its tiles don't wait
            # on phase-1 release ----
            with tc.tile_pool(name="ph23", bufs=1) as ph23:
                wp_all = ph23.tile([128, KC * D], F32R, tag="wp", name="wp_all")
                nc.sync.dma_start(
                    wp_all.rearrange("p (k t) -> p k t", k=KC),
                    wp.rearrange("(k p) t -> p k t", p=128),
                )
                wp_t = [wp_all[:, k * D : (k + 1) * D] for k in range(KC)]
                bp_sb = ph23.tile([1, D], F32R, tag="bp", name="bp_sb")
                nc.sync.dma_start(bp_sb[:], bp[:, :])
                yt_sb = [
                    ph23.tile([128, T], F32R, tag=f"yt{k}", name=f"yt{k}")
                    for k in range(KC)
                ]

                # ---- phase 1 + phase 2 interleaved: V first, then each head
                # pair right after its Q/K tiles so ACT exp starts early ----
                with tc.tile_pool(name="ph1", bufs=1) as ph1:
                    # x^T and W_v land in single big tiles via one strided DMA
                    # each: [k*128+p, t] -> [p, k*F + t]
                    # per-k DMAs so k=0 matmuls start after the first 512KB
                    xt_all = ph1.tile([128, KC * T], F32R, tag="xt", name="xt_all")
                    wv_all = ph1.tile([128, KC * VW], F32R, tag="wv", name="wv_all")
                    for k in range(KC):
                        for hh in range(2):
                            nc.sync.dma_start(
                                xt_all[:, k * T + hh * 512 : k * T + (hh + 1) * 512],
                                xT[k * 128 : (k + 1) * 128,
                                   hh * 512 : (hh + 1) * 512],
                            )
                        nc.sync.dma_start(
                            wv_all[:, k * VW : (k + 1) * VW],
                            wv[k * 128 : (k + 1) * 128, :],
                        )
                    xt = [xt_all[:, k * T : (k + 1) * T] for k in range(KC)]
                    wv_t = [wv_all[:, k * VW : (k + 1) * VW] for k in range(KC)]

                    for t in range(TC):
                        emit_v_chunk(t, xt, wv_t)

                    def load_pair_w(p):
                        t_w = ph1.tile([128, KC * 256], F32R, tag="wqk",
                                       bufs=2, name=f"wqkt{p}")
                        nc.sync.dma_start(
                            t_w.rearrange("p (k c) -> p k c", k=KC),
                            wqk[:, p * 256 : (p + 1) * 256].rearrange(
                                "(k q) c -> q k c", q=128
                            ),
                        )
                        return [t_w[:, k * 256 : (k + 1) * 256] for k in range(KC)]

                    # Q/K tiles computed >=1 pair ahead: prefetch pairs 0,1
                    # up front, then pair p+2 right after pair p completes
                    def prefetch(p):
                        if p < HP:
                            w_n = load_pair_w(p)
                            emit_qk_tile(p, xt, w_n, 0, 2 * p)
                            emit_qk_tile(KC + p, xt, w_n, 128, 2 * p + 1)

                    prefetch(0)
                    prefetch(1)
                    for hp_i in range(HP):
                        emit_pair(hp_i, yt_sb)
                        prefetch(hp_i + 2)

                for t in range(TC):
                    pr = ps.tile([128, D], F32, tag="sp", bufs=2, name=f"pr{t}")
                    for n0, n1 in ((0, 512), (512, D)):
                        nc.tensor.matmul(
                            pr[:, n0:n1],
                            _r(ones_sb),
                            _r(bp_sb[:, n0:n1]),
                            start=True,
                            stop=False,
                        )
                        for k in range(KC):
                            nc.tensor.matmul(
                                pr[:, n0:n1],
                                _r(yt_sb[k][:, t * 128 : (t + 1) * 128]),
                                _r(wp_t[k][:, n0:n1]),
                                start=False,
                                stop=(k == KC - 1),
                            )
                    o_t = ph23.tile([128, D], F32, tag="o", bufs=2, name=f"o{t}")
                    nc.vector.tensor_copy(o_t[:], pr[:])
                    nc.sync.dma_start(out[t * 128 : (t + 1) * 128, :], o_t[:])

    return nc


_NC = None


def _get_nc():
    global _NC
    if _NC is None:
        _NC = _build()
        _NC.finalize()
    return _NC


def _host_prep(x, W_qkv, b_qkv, W_proj, b_proj):
    x = np.ascontiguousarray(np.asarray(x, dtype=np.float32))
    W_qkv = np.asarray(W_qkv, dtype=np.float32)
    b_qkv = np.asarray(b_qkv, dtype=np.float32)
    W_proj = np.ascontiguousarray(np.asarray(W_proj, dtype=np.float32))
    b_proj = np.asarray(b_proj, dtype=np.float32)

    scale = np.float32(HD ** -0.5)  # 0.125, exact in fp32
    wq = (W_qkv[:, :D] * scale).astype(np.float32)
    wk = W_qkv[:, D : 2 * D].astype(np.float32)
    bq = (b_qkv[:D] * scale).astype(np.float32)
    bk = b_qkv[D : 2 * D].astype(np.float32)
    # per-pair blocks: [Wq_p | Wk_p] so each pair streams one [768,256] slab
    wqk_host = np.concatenate(
        sum(
            (
                [wq[:, p * 128 : (p + 1) * 128], wk[:, p * 128 : (p + 1) * 128]]
                for p in range(H // 2)
            ),
            [],
        ),
        axis=1,
    ).astype(np.float32)
    bqk_host = np.concatenate(
        sum(
            (
                [bq[p * 128 : (p + 1) * 128], bk[p * 128 : (p + 1) * 128]]
                for p in range(H // 2)
            ),
            [],
        )
    )[:, None].astype(np.float32)
    wv_host = np.zeros((D, VW), dtype=np.float32)
    bv_host = np.zeros((1, VW + 128), dtype=np.float32)
    bv_host[0, VW:] = 1.0
    for h in range(H):
        wv_host[:, h * 65 : h * 65 + 64] = W_qkv[:, 2 * D + h * 64 : 2 * D + (h + 1) * 64]
        bv_host[0, h * 65 : h * 65 + 64] = b_qkv[2 * D + h * 64 : 2 * D + (h + 1) * 64]
        bv_host[0, h * 65 + 64] = 1.0

    shared = {
        "wqk": np.ascontiguousarray(wqk_host),
        "bqk": np.ascontiguousarray(bqk_host),
        "wv": wv_host,
        "bv": bv_host,
        "wp": W_proj,
        "bp": np.ascontiguousarray(b_proj[None, :]),
    }
    in_maps = []
    for b in range(B):
        m = dict(shared)
        m["xT"] = np.ascontiguousarray(x[b].T)
        in_maps.append(m)
    return in_maps


def kernel(x, W_qkv, b_qkv, W_proj, b_proj):
    global LAST_RESULTS
    nc = _get_nc()
    in_maps = _host_prep(x, W_qkv, b_qkv, W_proj, b_proj)
    res = run_bass_kernel_spmd(nc, in_maps, list(range(B)), trace=TRACE)
    LAST_RESULTS = res
    return np.stack([res.results[b]["out"] for b in range(B)]).astype(np.float32)


# revision 21
# speedup vs baseline: 6.1742x; 6.1742x over previous
"""Multi-head self-attention (B=8, T=1024, D=768, H=12) on 8 Trainium2 NeuronCores.

Strategy: data-parallel over batch (1 batch element per core). Per core, all
tensors are kept in "feature-major" (transposed) layout so no on-chip
transposes are ever needed:

  phase 1:  Q^T = W_q^T @ x^T   [768,1024]   (Q pre-scaled by 1/sqrt(hd) on host)
            K^T = W_k^T @ x^T   [768,1024]
            V   = x @ W_v_aug   [1024, 12*65] natural layout, augmented with a
                  ones column per head (gives softmax denominator for free)
  phase 2:  per head pair (row-tiled on the PE array, 2 heads concurrently):
            S^T[k,q] = K^T_h.T(slice) @ Q^T_h   -> exp on ACT -> P^T
            y_aug^T = V_aug^T @ P^T  (row 64 = colsum of exp = softmax denom)
            y^T = y_unnorm^T * (1/colsum) broadcast via K=1 PE matmul
  phase 3:  out = y^T.T @ W_proj + b_proj  (bias seeded into PSUM via K=1
            ones-matmul), DMA PSUM -> DRAM.

All matmuls use the float32r dtype (full fp32 storage; PE processes it at
1 cycle/row when the moving free dim is >= 256, i.e. 4x faster than plain
fp32 matmul).
"""

import numpy as np

import concourse.bass as bass
import concourse.tile as tile
from concourse import bacc
from concourse import mybir
from concourse.bass_utils import run_bass_kernel_spmd

D = 768          # model dim
T = 1024         # sequence length
H = 12           # heads
HD = 64          # head dim
B = 8            # batch (== number of cores)
KC = D // 128    # 6 contraction chunks of 128
TC = T // 128    # 8 sequence chunks of 128
HP = H // 2      # 6 head pairs
VW = H * (HD + 1)  # 780: V augmented with a ones column per head

F32 = mybir.dt.float32
F32R = mybir.dt.float32r
EXP = mybir.ActivationFunctionType.Exp
ADD = mybir.AluOpType.add
MULT = mybir.AluOpType.mult

TRACE = False          # set True (from test harness) to collect an NTFF profile
LAST_RESULTS = None    # BassKernelResults of the most recent run (for test harness)


def _r(ap):
    return ap


def _build(reps=1):
    nc = bacc.Bacc("TRN2", target_bir_lowering=False, debug=False, num_devices=B)

    xT = nc.dram_tensor("xT", [D, T], F32R, kind="ExternalInput")
    wqk = nc.dram_tensor("wqk", [D, 2 * D], F32R, kind="ExternalInput")
    bqk = nc.dram_tensor("bqk", [2 * D, 1], F32, kind="ExternalInput")
    wv = nc.dram_tensor("wv", [D, VW], F32R, kind="ExternalInput")
    bv = nc.dram_tensor("bv", [1, VW + 128], F32R, kind="ExternalInput")
    wp = nc.dram_tensor("wp", [D, D], F32R, kind="ExternalInput")
    bp = nc.dram_tensor("bp", [1, D], F32R, kind="ExternalInput")
    out = nc.dram_tensor("out", [T, D], F32, kind="ExternalOutput")

    with tile.TileContext(nc) as tc:
      for _rep in range(reps):
        with (
            tc.tile_pool(name="persist", bufs=1) as persist,
            tc.tile_pool(name="work", bufs=1) as work,
            tc.tile_pool(name="ps", bufs=1, space="PSUM") as ps,
        ):
            # ---- persistent SBUF tensors (live through phases 1-2) ----
            qk_sb = [
                persist.tile([128, T], F32R, tag=f"qk{m}", name=f"qk{m}")
                for m in range(2 * KC)
            ]  # 0..5 = Q^T tiles, 6..11 = K^T tiles
            v_sb = [
                persist.tile([128, VW], F32R, tag=f"v{t}", name=f"v{t}")
                for t in range(TC)
            ]
            bqk_sb = persist.tile([128, 2 * KC], F32, tag="bqk", name="bqk_sb")
            bv_sb = persist.tile([1, VW + 128], F32R, tag="bv", name="bv_sb")
            ones_sb = bv_sb[:, VW : VW + 128]

            nc.sync.dma_start(bv_sb[:], bv[:, :])
            # all 12 bias columns in one strided DMA: bqk[m*128+p] -> [p, m]
            nc.sync.dma_start(
                bqk_sb[:, :],
                bqk.rearrange("(m p) 1 -> p m", p=128),
            )

            def emit_v_chunk(t, xt, wv_t):
                sv = ps.tile([128, VW], F32, tag="sp", bufs=2, name=f"sp_v{t}")
                for half, (n0, n1) in enumerate(((0, 512), (512, VW))):
                    nc.tensor.matmul(
                        sv[:, n0:n1],
                        _r(ones_sb),
                        _r(bv_sb[:, n0:n1]),
                        start=True,
                        stop=False,
                    )
                    for k in range(KC):
                        nc.tensor.matmul(
                            sv[:, n0:n1],
                            _r(xt[k][:, t * 128 : (t + 1) * 128]),
                            _r(wv_t[k][:, n0:n1]),
                            start=False,
                            stop=(k == KC - 1),
                        )
                nc.vector.tensor_copy(v_sb[t][:], sv[:])

            def emit_qk_tile(m, xt, w_t, col, bcol):
                # two 1-bank PSUM halves on the "pv" tag, which is free
                # between attention pairs -- keeps the "sp" ring a pure
                # PE<->ACT pipeline during the pair chunk loops
                for nh in range(2):
                    sp = ps.tile([128, 512], F32, tag="pv", bufs=4,
                                 name=f"sp_qk{m}_{nh}")
                    for k in range(KC):
                        nc.tensor.matmul(
                            sp[:, :],
                            _r(w_t[k][:, col : col + 128]),
                            _r(xt[k][:, nh * 512 : (nh + 1) * 512]),
                            start=(k == 0),
                            stop=(k == KC - 1),
                        )
                    nc.vector.tensor_scalar_add(
                        qk_sb[m][:, nh * 512 : (nh + 1) * 512],
                        sp[:],
                        bqk_sb[:, bcol : bcol + 1],
                    )

            def emit_pair(hp_i, yt_sb):
                qt = qk_sb[hp_i]
                kt = qk_sb[KC + hp_i]
                pv = {}
                for hi in range(2):
                    for half in range(2):
                        pv[hi, half] = ps.tile(
                            [65, 512], F32, tag="pv", bufs=4,
                            name=f"pv{hp_i}_{hi}_{half}",
                        )
                for kb in range(TC):
                    sps = [
                        ps.tile([128, T], F32, tag="sp", bufs=2,
                                name=f"sp_s{hp_i}_{kb}_{hi}")
                        for hi in range(2)
                    ]
                    # QK^T: two heads concurrently in disjoint 64-row strips
                    for half in range(2):
                        for hi in range(2):
                            r0, r1 = hi * 64, (hi + 1) * 64
                            nc.tensor.matmul(
                                sps[hi][:, half * 512 : (half + 1) * 512],
                                _r(kt[r0:r1, kb * 128 : (kb + 1) * 128]),
                                _r(qt[r0:r1, half * 512 : (half + 1) * 512]),
                                start=True,
                                stop=True,
                                tile_position=(hi * 64, 0),
                            )
                    for hi in range(2):
                        h = 2 * hp_i + hi
                        p_t = work.tile([128, T], F32R, tag="p", bufs=4,
                                        name=f"p{hp_i}_{kb}_{hi}")
                        nc.scalar.activation(p_t[:], sps[hi][:], EXP)
                        for half in range(2):
                            nc.tensor.matmul(
                                pv[hi, half][:, :],
                                _r(v_sb[kb][:, h * 65 : (h + 1) * 65]),
                                _r(p_t[:, half * 512 : (half + 1) * 512]),
                                start=(kb == 0),
                                stop=(kb == TC - 1),
                            )
                # normalize: y^T[f, q] * (1/colsum[q]) and write into y^T sbuf.
                # Single copy frees the pv PSUM slot immediately; the rest of
                # the chain runs off SBUF and overlaps the next pair.
                for hi in range(2):
                    for half in range(2):
                        yc = work.tile([65, 512], F32, tag="yc", bufs=3,
                                       name=f"yc{hp_i}_{hi}_{half}")
                        nc.vector.tensor_copy(yc[:], pv[hi, half][:, :])
                        rc = work.tile([1, 512], F32R, tag="rc", bufs=2,
                                       name=f"rc{hp_i}_{hi}_{half}")
                        with nc.allow_low_precision(reason="f32r rounding of softmax denom"):
                            nc.vector.reciprocal(rc[:], yc[64:65, :])
                        rb = ps.tile([64, 512], F32, tag="pv", bufs=4,
                                     name=f"rb{hp_i}_{hi}_{half}")
                        nc.tensor.matmul(
                            rb[:, :],
                            _r(ones_sb[:, 0:64]),
                            _r(rc[:, :]),
                            start=True,
                            stop=True,
                        )
                        nc.vector.tensor_tensor(
                            yt_sb[hp_i][hi * 64 : (hi + 1) * 64,
                                        half * 512 : (half + 1) * 512],
                            yc[0:64, :],
                            rb[:, :],
                            op=MULT,
                        )

            # ---- phases 2+3 pool opened before ph1 so its tiles don't wait
            # on phase-1 release ----
            with tc.tile_pool(name="ph23", bufs=1) as ph23:
                wp_all = ph23.tile([128, KC * D], F32R, tag="wp", name="wp_all")
                nc.sync.dma_start(
                    wp_all.rearrange("p (k t) -> p k t", k=KC),
                    wp.rearrange("(k p) t -> p k t", p=128),
                )
                wp_t = [wp_all[:, k * D : (k + 1) * D] for k in range(KC)]
                bp_sb = ph23.tile([1, D], F32R, tag="bp", name="bp_sb")
                nc.sync.dma_start(bp_sb[:], bp[:, :])
                yt_sb = [
                    ph23.tile([128, T], F32R, tag=f"yt{k}", name=f"yt{k}")
                    for k in range(KC)
                ]

                # ---- phase 1 + phase 2 interleaved: V first, then each head
                # pair right after its Q/K tiles so ACT exp starts early ----
                with tc.tile_pool(name="ph1", bufs=1) as ph1:
                    # x^T and W_v land in single big tiles via one strided DMA
                    # each: [k*128+p, t] -> [p, k*F + t]
                    # per-k DMAs so k=0 matmuls start after the first 512KB
                    xt_all = ph1.tile([128, KC * T], F32R, tag="xt", name="xt_all")
                    wv_all = ph1.tile([128, KC * VW], F32R, tag="wv", name="wv_all")
                    for k in range(KC):
                        for hh in range(2):
                            nc.sync.dma_start(
                                xt_all[:, k * T + hh * 512 : k * T + (hh + 1) * 512],
                                xT[k * 128 : (k + 1) * 128,
                                   hh * 512 : (hh + 1) * 512],
                            )
                        nc.sync.dma_start(
                            wv_all[:, k * VW : (k + 1) * VW],
                            wv[k * 128 : (k + 1) * 128, :],
                        )
                    xt = [xt_all[:, k * T : (k + 1) * T] for k in range(KC)]
                    wv_t = [wv_all[:, k * VW : (k + 1) * VW] for k in range(KC)]

                    for t in range(TC):
                        emit_v_chunk(t, xt, wv_t)

                    def load_pair_w(p):
                        t_w = ph1.tile([128, KC * 256], F32R, tag="wqk",
                                       bufs=2, name=f"wqkt{p}")
                        nc.sync.dma_start(
                            t_w.rearrange("p (k c) -> p k c", k=KC),
                            wqk[:, p * 256 : (p + 1) * 256].rearrange(
                                "(k q) c -> q k c", q=128
                            ),
                        )
                        return [t_w[:, k * 256 : (k + 1) * 256] for k in range(KC)]

                    # Q/K tiles computed >=1 pair ahead: prefetch pairs 0,1
                    # up front, then pair p+2 right after pair p completes
                    def prefetch(p):
                        if p < HP:
                            w_n = load_pair_w(p)
                            emit_qk_tile(p, xt, w_n, 0, 2 * p)
                            emit_qk_tile(KC + p, xt, w_n, 128, 2 * p + 1)

                    prefetch(0)
                    prefetch(1)
                    for hp_i in range(HP):
                        emit_pair(hp_i, yt_sb)
                        prefetch(hp_i + 2)

                for t in range(TC):
                    pr = ps.tile([128, D], F32, tag="sp", bufs=2, name=f"pr{t}")
                    for n0, n1 in ((0, 512), (512, D)):
                        nc.tensor.matmul(
                            pr[:, n0:n1],
                            _r(ones_sb),
                            _r(bp_sb[:, n0:n1]),
                            start=True,
                            stop=False,
                        )
                        for k in range(KC):
                            nc.tensor.matmul(
                                pr[:, n0:n1],
                                _r(yt_sb[k][:, t * 128 : (t + 1) * 128]),
                                _r(wp_t[k][:, n0:n1]),
                                start=False,
                                stop=(k == KC - 1),
                            )
                    o_t = ph23.tile([128, D], F32, tag="o", bufs=2, name=f"o{t}")
                    nc.vector.tensor_copy(o_t[:], pr[:])
                    nc.sync.dma_start(out[t * 128 : (t + 1) * 128, :], o_t[:])

    return nc


_NC = None


def _get_nc():
    global _NC
    if _NC is None:
        _NC = _build()
        _NC.finalize()
    return _NC


def _host_prep(x, W_qkv, b_qkv, W_proj, b_proj):
    x = np.ascontiguousarray(np.asarray(x, dtype=np.float32))
    W_qkv = np.asarray(W_qkv, dtype=np.float32)
    b_qkv = np.asarray(b_qkv, dtype=np.float32)
    W_proj = np.ascontiguousarray(np.asarray(W_proj, dtype=np.float32))
    b_proj = np.asarray(b_proj, dtype=np.float32)

    scale = np.float32(HD ** -0.5)  # 0.125, exact in fp32
    wq = (W_qkv[:, :D] * scale).astype(np.float32)
    wk = W_qkv[:, D : 2 * D].astype(np.float32)
    bq = (b_qkv[:D] * scale).astype(np.float32)
    bk = b_qkv[D : 2 * D].astype(np.float32)
    # per-pair blocks: [Wq_p | Wk_p] so each pair streams one [768,256] slab
    wqk_host = np.concatenate(
        sum(
            (
                [wq[:, p * 128 : (p + 1) * 128], wk[:, p * 128 : (p + 1) * 128]]
                for p in range(H // 2)
            ),
            [],
        ),
        axis=1,
    ).astype(np.float32)
    bqk_host = np.concatenate(
        sum(
            (
                [bq[p * 128 : (p + 1) * 128], bk[p * 128 : (p + 1) * 128]]
                for p in range(H // 2)
            ),
            [],
        )
    )[:, None].astype(np.float32)
    wv_host = np.zeros((D, VW), dtype=np.float32)
    bv_host = np.zeros((1, VW + 128), dtype=np.float32)
    bv_host[0, VW:] = 1.0
    for h in range(H):
        wv_host[:, h * 65 : h * 65 + 64] = W_qkv[:, 2 * D + h * 64 : 2 * D + (h + 1) * 64]
        bv_host[0, h * 65 : h * 65 + 64] = b_qkv[2 * D + h * 64 : 2 * D + (h + 1) * 64]
        bv_host[0, h * 65 + 64] = 1.0

    shared = {
        "wqk": np.ascontiguousarray(wqk_host),
        "bqk": np.ascontiguousarray(bqk_host),
        "wv": wv_host,
        "bv": bv_host,
        "wp": W_proj,
        "bp": np.ascontiguousarray(b_proj[None, :]),
    }
    in_maps = []
    for b in range(B):
        m = dict(shared)
        m["xT"] = np.ascontiguousarray(x[b].T)
        in_maps.append(m)
    return in_maps


def kernel(x, W_qkv, b_qkv, W_proj, b_proj):
    global LAST_RESULTS
    nc = _get_nc()
    in_maps = _host_prep(x, W_qkv, b_qkv, W_proj, b_proj)
    res = run_bass_kernel_spmd(nc, in_maps, list(range(B)), trace=TRACE)
    LAST_RESULTS = res
    return np.stack([res.results[b]["out"] for b in range(B)]).astype(np.float32)
